# revision 50
# baseline (speedup 1.0000x reference)
"""LoRA generator kernel for Trainium2, sharded over 8 NeuronCores by layer.

Reference computation (see problem):
  pe = (condition @ W_proj + b_proj)                        (B=2, 224, 512)
  A  = (gelu(pe@WA1+bA1) @ WA2 + bA2) -> (B, L, 7, 16, 64)
  Bm = (gelu(pe@WB1+bB1) @ WB2 + bB2) -> (B, L, 7, 64, 16)
  out per (b, layer): concat over t of [tile_cols(A)*scA (16 x in_d),
                                        tile_rows(B)*scB (out_d x 16)]

Each core handles 4 layers (28 of the 224 projections). The big costs are
streaming its W_proj slice (11MB in bf16) and writing its 36.8MB output
slice. Pipeline: 4 rounds, one layer each — round r loads layer r's W_proj
columns (one 2.75MB DMA), computes pe, decodes, and drains the layer's
output while round r+1 loads.

Output path (SDMA engine = partition-iteration index mod 16; SBUF port =
partition//4, so stride-7 partition sets cycle through all 16 ports):
  A pieces: decoder rows are scattered to partition 7*rank + slot, the
    64-float chunk is doubled up to 4096 wide (sync-queue DMA bridges the
    raw-AP scatters into tracked copy-chain deps, split DVE/ACT), then each
    piece is written with 16KB descriptors, one per rank-row.
  B pieces: mm2 runs with the per-row h vector duplicated 16x in the
    stationary free dim, so each row's 4KB block lands replicated on 16
    stride-7 partitions; after a column duplication (8KB runs) the piece is
    written straight out with 8KB descriptors over all 16 engines.
"""
import sys

sys.path.insert(0, "/opt/trn_rl_repo")

import numpy as np
import ml_dtypes

import concourse.bass as bass
import concourse.bacc as bacc
import concourse.mybir as mybir
import concourse.tile as tile
from concourse.bass_utils import run_bass_kernel_spmd

F32 = mybir.dt.float32
BF16 = mybir.dt.bfloat16
ACT_FN = mybir.ActivationFunctionType.Gelu  # sim override hook
NPBF16 = ml_dtypes.bfloat16

NCORES = 8
NUM_LAYERS = 32
RANK = 16
PED = 512
EMB = 384
T = 7
L = NUM_LAYERS // NCORES          # 4 layers per core
LT = L * T                        # 28 projections per core
ROWS = 2 * LT                     # 56 rows (b, l, t); row = (l*7+t)*2 + b
WP_COLS = LT * PED                # 14336
RPL = 2 * T                       # 14 rows per layer

IN_DS = [4096, 4096, 4096, 4096, 4096, 4096, 11008]
OUT_DS = [4096, 1024, 1024, 4096, 11008, 11008, 4096]
A_SIZES = [16 * d for d in IN_DS]
B_SIZES = [16 * d for d in OUT_DS]
LAYER_SIZE = sum(A_SIZES) + sum(B_SIZES)   # 1150976
OFF_A = []
OFF_B = []
_o = 0
for _t in range(T):
    OFF_A.append(_o)
    _o += A_SIZES[_t]
    OFF_B.append(_o)
    _o += B_SIZES[_t]
OUT_SZ = 2 * L * LAYER_SIZE

RCOLS = T * PED                   # 3584 W_proj columns per round (1 layer)

PB_L = [0, 32, 64, 64]           # partition base per layer (engine ops need 0/32/64)
ACOL = [0, 0, 0, 1024]           # oa column offset per layer
BCOL = [0, 0, 0, 1024]           # ob / scaled-bias column offset per layer

# slot -> (t, b) within a group; even groups hold rows 0-6 of the layer,
# odd groups rows 7-13, where row = 2*t + b.
SLOT_TB = [
    [(0, 0), (0, 1), (1, 0), (1, 1), (2, 0), (2, 1), (3, 0)],
    [(3, 1), (4, 0), (4, 1), (5, 0), (5, 1), (6, 0), (6, 1)],
]

# B piece layout: piece i = 2t+b owns partition group 16*(i%8)..+16 and
# column (i//8)*1024 of the layer's bexp buffer; its 4KB block is replicated
# on all 16 group partitions so the out DMA's descriptors (engine = partition
# iteration index mod 16) spread over all 16 SDMA engines.
NB = [d // 64 for d in OUT_DS]     # 4KB chunks per piece


def _gbase(g):
    """First partition of row-group g (7 rows each, g = 2*l + parity)."""
    return PB_L[g // 2] + 7 * (g % 2)


def _build_nc():
    nc = bacc.Bacc(None, target_bir_lowering=False, debug=False)

    cond = nc.declare_dram_parameter("cond", [128, 6], BF16, isOutput=False)
    wp = nc.declare_dram_parameter("wp", [EMB, WP_COLS], BF16, isOutput=False)
    bpt2 = nc.declare_dram_parameter("bpt2", [128, 4 * ROWS], F32, isOutput=False)
    wa1 = nc.declare_dram_parameter("wa1", [128, 1024], BF16, isOutput=False)
    wb1 = nc.declare_dram_parameter("wb1", [128, 1024], BF16, isOutput=False)
    wa2 = nc.declare_dram_parameter("wa2", [128, 2048], BF16, isOutput=False)
    wb2 = nc.declare_dram_parameter("wb2", [128, 2048], BF16, isOutput=False)
    ba1 = nc.declare_dram_parameter("ba1", [128, 2], F32, isOutput=False)
    bb1 = nc.declare_dram_parameter("bb1", [128, 2], F32, isOutput=False)
    sca = nc.declare_dram_parameter("sca", [128, ROWS], BF16, isOutput=False)
    scb = nc.declare_dram_parameter("scb", [128, ROWS], BF16, isOutput=False)
    sba2 = nc.declare_dram_parameter("sba2", [128, 2048], BF16, isOutput=False)
    # pre-scaled bB2 bias in the replicated B layout (partition = dup*7 +
    # row%7, column block = layer), one tile per row-half
    sbb2r0 = nc.declare_dram_parameter("sbb2r0", [128, 4 * 1024], BF16, isOutput=False)
    sbb2r1 = nc.declare_dram_parameter("sbb2r1", [128, 4 * 1024], BF16, isOutput=False)
    ident = nc.declare_dram_parameter("ident", [128, 2], F32, isOutput=False)
    out = nc.declare_dram_parameter("out", [OUT_SZ], F32, isOutput=True)

    with tile.TileContext(nc) as tc:
        with (
            tc.tile_pool(name="const", bufs=1) as cpool,
            tc.tile_pool(name="wp", bufs=2) as wpool,
            tc.tile_pool(name="work", bufs=1) as wkpool,
            tc.tile_pool(name="pe2", bufs=2) as pe2pool,
            tc.tile_pool(name="ps", bufs=1, space="PSUM") as ps,
        ):
            cond_sb = cpool.tile([128, 6], BF16)
            nc.sync.dma_start(cond_sb[:], cond[:])
            # round-0 W_proj load on the sync queue (HWDGE): issued before
            # the consts so layer 0's pe compute starts as early as possible
            wp0_t = wpool.tile([128, 3 * RCOLS], BF16, tag="wp", name="wp0")
            pwt0 = wp0_t[:, :].ap[0][0]
            for c0_, cw in ((0, 2 * PED), (2 * PED, 2 * PED), (4 * PED, RCOLS - 4 * PED)):
                wp_src = bass.AP(
                    wp, c0_, [[WP_COLS, 128], [128 * WP_COLS, 3], [1, cw]]
                )
                wp_dst = bass.AP(
                    wp0_t[:, :].tensor, c0_, [[pwt0, 128], [RCOLS, 3], [1, cw]]
                )
                nc.sync.dma_start(wp_dst, wp_src)
            bpt2_sb = cpool.tile([128, 4 * ROWS], F32)
            nc.sync.dma_start(bpt2_sb[:], bpt2[:])
            wa1_sb = cpool.tile([128, 1024], BF16)
            nc.sync.dma_start(wa1_sb[:], wa1[:])
            wb1_sb = cpool.tile([128, 1024], BF16)
            nc.sync.dma_start(wb1_sb[:], wb1[:])
            wa2_sb = cpool.tile([128, 2048], BF16)
            nc.sync.dma_start(wa2_sb[:], wa2[:])
            wb2_sb = cpool.tile([128, 2048], BF16)
            nc.sync.dma_start(wb2_sb[:], wb2[:])
            ba1_sb = cpool.tile([128, 2], F32)
            nc.sync.dma_start(ba1_sb[:], ba1[:])
            bb1_sb = cpool.tile([128, 2], F32)
            nc.sync.dma_start(bb1_sb[:], bb1[:])
            sca_sb = cpool.tile([128, ROWS], BF16)
            nc.sync.dma_start(sca_sb[:], sca[:])
            scb_sb = cpool.tile([128, ROWS], BF16)
            nc.sync.dma_start(scb_sb[:], scb[:])
            sba2_sb = cpool.tile([128, 2048], BF16)
            nc.scalar.dma_start(sba2_sb[:], sba2[:])
            sbb2r_sb = []
            for hi, prm in enumerate((sbb2r0, sbb2r1)):
                t_ = cpool.tile(
                    [128, 4 * 1024], BF16, tag=f"sbb2r{hi}", name=f"sbb2r{hi}"
                )
                nc.scalar.dma_start(t_[:], prm[:])
                sbb2r_sb.append(t_)
            ident_sb = cpool.tile([128, 2], F32)
            nc.sync.dma_start(ident_sb[:], ident[:])

            # long-lived work tiles
            pe_sb = [
                wkpool.tile([128, ROWS], BF16, tag=f"pe_sb{mc}", name=f"pe_sb{mc}")
                for mc in range(4)
            ]
            oa = wkpool.tile([128, 2048], F32)     # decoder A out
            pa = oa[:, :].ap[0][0]
            oa_t = oa[:, :].tensor
            # expansion buffers, rotated manually (layer l+k reuses after
            # the piece DMAs of layer l drained; Tile tracks the WAR deps)
            aexp_bufs = [
                wkpool.tile([128, 4096], F32, tag=f"aexpb{i}", name=f"aexpb{i}")
                for i in range(4)
            ]
            # replicated B decoder outputs: obh[half][layer%2] holds, on
            # partition dup*7 + row, the row's 4KB block twice (8KB runs)
            obh = [
                [
                    wkpool.tile(
                        [128, 2048], F32, tag=f"obh{h}{p}", name=f"obh{h}{p}"
                    )
                    for p in range(3)
                ]
                for h in range(2)
            ]
            anchor_sb = wkpool.tile([128, 16], F32, tag="anchor", name="anchor_sb")

            def decode_layer(l):
                """Decoder MLPs + expansion + piece DMAs for layer l."""
                c0 = RPL * l              # first row / pe_sb column of the layer
                pb = PB_L[l]              # partition base (0/32/64)
                acol, bcol = ACOL[l], BCOL[l]
                # A decoder: h pre-scaled by sca, pre-scaled bias added
                ha_sb = []
                for mc in range(2):
                    hp = ps.tile([128, RPL], F32, tag=f"h{mc}", name=f"hp{mc}")
                    for kc in range(4):
                        nc.tensor.matmul(
                            hp[:],
                            wa1_sb[:, kc * 256 + mc * 128 : kc * 256 + (mc + 1) * 128],
                            pe_sb[kc][:, c0 : c0 + RPL],
                            start=(kc == 0),
                            stop=(kc == 3),
                        )
                    hs = wkpool.tile(
                        [128, RPL], BF16, tag=f"h_sb0{mc}", name=f"hsa{mc}"
                    )
                    nc.scalar.activation(
                        hs[:], hp[:], ACT_FN, bias=ba1_sb[:, mc : mc + 1]
                    )
                    nc.vector.tensor_mul(hs[:], hs[:], sca_sb[:, c0 : c0 + RPL])
                    ha_sb.append(hs)
                for nh in range(2):
                    op = ps.tile([128, 512], F32, tag=f"o{nh}", name=f"op{nh}")
                    for kc in range(2):
                        nc.tensor.matmul(
                            op[pb : pb + RPL, :],
                            ha_sb[kc][:],
                            wa2_sb[:, kc * 1024 + nh * 512 : kc * 1024 + (nh + 1) * 512],
                            start=(kc == 0),
                            stop=(kc == 1),
                        )
                    nc.vector.tensor_add(
                        oa[pb : pb + RPL, acol + nh * 512 : acol + (nh + 1) * 512],
                        op[pb : pb + RPL, :],
                        sba2_sb[pb : pb + RPL, bcol + nh * 512 : bcol + (nh + 1) * 512],
                    )

                # B decoder: rows are replicated 16x through mm2 (duplicated
                # h in the stationary free dim), bias + per-row scale applied
                # after, so the piece DMAs read 16 partitions directly
                hb_sb = []
                for mc in range(2):
                    hp = ps.tile([128, RPL], F32, tag=f"h{mc}", name=f"hpb{mc}")
                    for kc in range(4):
                        nc.tensor.matmul(
                            hp[:],
                            wb1_sb[:, kc * 256 + mc * 128 : kc * 256 + (mc + 1) * 128],
                            pe_sb[kc][:, c0 : c0 + RPL],
                            start=(kc == 0),
                            stop=(kc == 3),
                        )
                    hs = wkpool.tile(
                        [128, RPL], BF16, tag=f"h_sb1{mc}", name=f"hsb{mc}"
                    )
                    nc.scalar.activation(
                        hs[:], hp[:], ACT_FN, bias=bb1_sb[:, mc : mc + 1]
                    )
                    nc.vector.tensor_mul(hs[:], hs[:], scb_sb[:, c0 : c0 + RPL])
                    hb_sb.append(hs)
                for half in range(2):
                    hd = []
                    for kc in range(2):
                        hdt = wkpool.tile(
                            [128, 112], BF16, tag=f"hd{kc}{half}",
                            name=f"hd{kc}{half}",
                        )
                        nc.vector.tensor_copy(
                            hdt[:, 0:7],
                            hb_sb[kc][:, 7 * half : 7 * half + 7],
                        )
                        w = 7
                        while w < 112:
                            nc.vector.tensor_copy(
                                hdt[:, w : 2 * w], hdt[:, 0:w]
                            )
                            w *= 2
                        hd.append(hdt)
                    tgt = obh[half][l % 3]
                    for nh in range(2):
                        op = ps.tile([128, 512], F32, tag=f"o{nh}", name=f"opb{nh}")
                        for kc in range(2):
                            nc.tensor.matmul(
                                op[0:112, :],
                                hd[kc][:, 0:112],
                                wb2_sb[:, kc * 1024 + nh * 512 : kc * 1024 + (nh + 1) * 512],
                                start=(kc == 0),
                                stop=(kc == 1),
                            )
                        nc.vector.tensor_add(
                            tgt[0:112, nh * 512 : (nh + 1) * 512],
                            op[0:112, :],
                            sbb2r_sb[half][
                                0:112, l * 1024 + nh * 512 : l * 1024 + (nh + 1) * 512
                            ],
                        )
                    # duplicate the 4KB block to make 8KB contiguous runs
                    nc.scalar.copy(tgt[0:112, 1024:2048], tgt[0:112, 0:1024])

                # ---- A pieces ----
                # aexp layout: partition 7*rank + slot (stride-7 sources
                # cycle through all 16 SBUF ports; iteration length 16
                # spreads every DMA over all 16 engines). Ordering relies on
                # sync-queue FIFO: scatters -> double1 -> (copy chain) ->
                # anchor -> piece DMAs, with the copy chain bridged by
                # slice-AP DMAs on both sides.
                for g in (2 * l, 2 * l + 1):
                    gb = _gbase(g)
                    aexp = aexp_bufs[g % 4]
                    aexp_t = aexp[:, :].tensor
                    pax = aexp[:, :].ap[0][0]
                    # even groups ride the sync queue, odd groups the gpsimd
                    # queue: group g's buffer-reuse WAR partner is g-4 (same
                    # parity, same queue -> FIFO keeps the ordering), and odd
                    # groups' scatters no longer queue behind even groups'
                    # piece drains
                    qe = nc.sync if g % 2 == 0 else nc.gpsimd
                    # scatter: aexp[7r+s, 0:64] = oa[gb+s, acol+64r : +64]
                    for s in range(7):
                        dst = bass.AP(aexp_t, s * pax, [[7 * pax, 16], [1, 64]])
                        qe.dma_start(
                            dst, oa[gb + s : gb + s + 1, acol : acol + 1024]
                        )
                    # first doubling (slice APs bridge the raw-AP scatters
                    # into tracked engine-op dependencies)
                    qe.dma_start(aexp[0:112, 64:128], aexp[0:112, 0:64])
                for g in (2 * l, 2 * l + 1):
                    aexp = aexp_bufs[g % 4]
                    w = 128
                    while w < 4096:
                        if g % 2 == 0:
                            nc.vector.tensor_copy(
                                aexp[0:112, w : 2 * w], aexp[0:112, 0:w]
                            )
                        else:
                            nc.scalar.copy(
                                aexp[0:112, w : 2 * w], aexp[0:112, 0:w]
                            )
                        w *= 2
                for g in (2 * l, 2 * l + 1):
                    gb = _gbase(g)
                    aexp = aexp_bufs[g % 4]
                    aexp_t = aexp[:, :].tensor
                    pax = aexp[:, :].ap[0][0]
                    qe = nc.sync if g % 2 == 0 else nc.gpsimd
                    # anchor: tracked read of the copy-chain tail so the
                    # piece DMAs (FIFO behind it) start after the expansion
                    qe.dma_start(
                        anchor_sb[0:112, g % 8 : g % 8 + 1],
                        aexp[0:112, 4095:4096],
                    )
                    for s in range(7):
                        t, b = SLOT_TB[g % 2][s]
                        in_d = IN_DS[t]
                        base = (b * L + l) * LAYER_SIZE + OFF_A[t]
                        if in_d == 4096:
                            dstp = bass.AP(out, base, [[4096, 16], [1, 4096]])
                            srcp = bass.AP(
                                aexp_t, s * pax, [[7 * pax, 16], [1, 4096]]
                            )
                            qe.dma_start(dstp, srcp)
                        else:  # 11008 = 2*4096 + 2816
                            dstp = bass.AP(
                                out, base, [[in_d, 16], [4096, 2], [1, 4096]]
                            )
                            srcp = bass.AP(
                                aexp_t, s * pax,
                                [[7 * pax, 16], [0, 2], [1, 4096]],
                            )
                            qe.dma_start(dstp, srcp)
                            dstp = bass.AP(
                                out, base + 8192, [[in_d, 16], [1, 2816]]
                            )
                            srcp = bass.AP(
                                aexp_t, s * pax, [[7 * pax, 16], [1, 2816]]
                            )
                            qe.dma_start(dstp, srcp)

                # ---- B pieces ----
                # row (t, b) lives replicated on partitions {i' + 7d} of
                # obh[half]; the out DMA iterates the 16 replicas (16
                # engines, stride-7 ports), 8KB descriptors
                for half in range(2):
                    tgt = obh[half][l % 3]
                    tgt_t = tgt[:, :].tensor
                    pobh = tgt[:, :].ap[0][0]
                    # half 0 drains on the scalar queue, half 1 on sync: the
                    # buffer-reuse WAR partner (same half, l+2) stays on the
                    # same queue, and the per-queue backlog halves
                    qb = nc.scalar if half == 0 else nc.sync
                    # anchor: tracked read of the duplicated tail so the
                    # piece DMAs (FIFO behind it) wait for the DVE chain
                    qb.dma_start(
                        anchor_sb[:, 2 * (l % 3) + half : 2 * (l % 3) + half + 1],
                        tgt[:, 2047:2048],
                    )
                    for i_ in range(7):
                        i = 7 * half + i_
                        t, b = i // 2, i % 2
                        nb2 = NB[t] // 2          # 8KB chunks per piece
                        nq, nr = nb2 // 16, nb2 % 16
                        base = (b * L + l) * LAYER_SIZE + OFF_B[t]
                        if nq:
                            dstp = bass.AP(
                                out, base,
                                [[nq * 2048, 16], [2048, nq], [1, 2048]],
                            )
                            srcp = bass.AP(
                                tgt_t, i_ * pobh,
                                [[7 * pobh, 16], [0, nq], [1, 2048]],
                            )
                            qb.dma_start(dstp, srcp)
                        if nr:
                            dstp = bass.AP(
                                out, base + 16 * nq * 2048,
                                [[2048, nr], [1, 2048]],
                            )
                            srcp = bass.AP(
                                tgt_t, i_ * pobh,
                                [[7 * pobh, nr], [1, 2048]],
                            )
                            qb.dma_start(dstp, srcp)

            # ---- main pipeline: one layer per round ----
            for rd in range(L):
                if rd == 0:
                    wp_t = wp0_t
                else:
                    wp_t = wpool.tile(
                        [128, 3 * RCOLS], BF16, tag="wp", name=f"wp{rd}"
                    )
                    pwt = wp_t[:, :].ap[0][0]
                    wp_src = bass.AP(
                        wp, rd * RCOLS,
                        [[WP_COLS, 128], [128 * WP_COLS, 3], [1, RCOLS]],
                    )
                    wp_dst = bass.AP(
                        wp_t[:, :].tensor, 0, [[pwt, 128], [RCOLS, 3], [1, RCOLS]]
                    )
                    nc.gpsimd.dma_start(wp_dst, wp_src)
                c0r = RPL * rd
                tr_all = ps.tile([128, 64], F32, tag="tra", name="tr_all")
                for ltl in range(T):
                    lt = rd * T + ltl
                    p2 = ps.tile([2, PED], F32, tag=f"p2{ltl % 2}", name="pe2_ps")
                    for kc in range(3):
                        nc.tensor.matmul(
                            p2[:],
                            cond_sb[:, kc * 2 : kc * 2 + 2],
                            wp_t[:, kc * RCOLS + ltl * PED : kc * RCOLS + (ltl + 1) * PED],
                            start=(kc == 0),
                            stop=(kc == 2),
                        )
                    pe2_sb = pe2pool.tile(
                        [2, PED], F32, tag=f"pe2sb{ltl % 2}", name="pe2_sb"
                    )
                    nc.vector.tensor_copy(pe2_sb[:], p2[:])
                    for mc in range(4):
                        nc.tensor.transpose(
                            tr_all[:, mc * 16 + 2 * ltl : mc * 16 + 2 * ltl + 2],
                            pe2_sb[:, mc * 128 : (mc + 1) * 128],
                            ident_sb[0:2, 0:2],
                        )
                # pe_T with b_proj bias, one batched add per PED-chunk
                for mc in range(4):
                    nc.vector.tensor_add(
                        pe_sb[mc][:, c0r : c0r + RPL],
                        tr_all[:, mc * 16 : mc * 16 + RPL],
                        bpt2_sb[:, mc * ROWS + c0r : mc * ROWS + c0r + RPL],
                    )
                decode_layer(rd)

    nc.finalize()
    return nc


_NC = None


def _get_nc():
    global _NC
    if _NC is None:
        _NC = _build_nc()
    return _NC


def _marshal(inputs):
    """Build the per-core input maps from full inputs."""
    condition = np.asarray(inputs["condition"], np.float32)
    W_proj = np.asarray(inputs["W_proj"], np.float32)
    b_proj = np.asarray(inputs["b_proj"], np.float32)
    WA1 = np.asarray(inputs["WA1"], np.float32)
    bA1 = np.asarray(inputs["bA1"], np.float32)
    WA2 = np.asarray(inputs["WA2"], np.float32)
    bA2 = np.asarray(inputs["bA2"], np.float32)
    WB1 = np.asarray(inputs["WB1"], np.float32)
    bB1 = np.asarray(inputs["bB1"], np.float32)
    WB2 = np.asarray(inputs["WB2"], np.float32)
    bB2 = np.asarray(inputs["bB2"], np.float32)
    scales = np.asarray(inputs["scales"], np.float32)

    cond_arr = np.zeros((128, 6), np.float32)
    for kc in range(3):
        cond_arr[:, kc * 2 : kc * 2 + 2] = condition[:, kc * 128 : (kc + 1) * 128].T
    cond_arr = cond_arr.astype(NPBF16)
    wa1_arr = np.zeros((128, 1024), np.float32)
    wb1_arr = np.zeros((128, 1024), np.float32)
    for kc in range(4):
        wa1_arr[:, kc * 256 : (kc + 1) * 256] = WA1[kc * 128 : (kc + 1) * 128, :]
        wb1_arr[:, kc * 256 : (kc + 1) * 256] = WB1[kc * 128 : (kc + 1) * 128, :]
    wa2_arr = np.zeros((128, 2048), np.float32)
    wb2_arr = np.zeros((128, 2048), np.float32)
    for kc in range(2):
        wa2_arr[:, kc * 1024 : (kc + 1) * 1024] = WA2[kc * 128 : (kc + 1) * 128, :]
        wb2_arr[:, kc * 1024 : (kc + 1) * 1024] = WB2[kc * 128 : (kc + 1) * 128, :]
    wa1_arr = wa1_arr.astype(NPBF16)
    wb1_arr = wb1_arr.astype(NPBF16)
    wa2_arr = wa2_arr.astype(NPBF16)
    wb2_arr = wb2_arr.astype(NPBF16)
    ba1_arr = np.ascontiguousarray(bA1.reshape(2, 128).T)
    bb1_arr = np.ascontiguousarray(bB1.reshape(2, 128).T)
    ident_arr = np.zeros((128, 2), np.float32)
    ident_arr[0, 0] = 1.0
    ident_arr[1, 1] = 1.0

    in_maps = []
    for c in range(NCORES):
        lt0 = c * LT
        wp_c = np.ascontiguousarray(
            W_proj[:, lt0 * PED : (lt0 + LT) * PED]
        ).astype(NPBF16)
        bp_c = b_proj[lt0 * PED : (lt0 + LT) * PED].reshape(LT, 4, 128)
        bpt2_arr = np.zeros((128, 4 * ROWS), np.float32)
        for row in range(ROWS):
            for mc in range(4):
                bpt2_arr[:, mc * ROWS + row] = bp_c[row // 2, mc, :]
        sca_row = np.zeros(ROWS, np.float32)
        scb_row = np.zeros(ROWS, np.float32)
        for row in range(ROWS):
            lt = row // 2
            sca_row[row] = scales[lt0 + lt, 0]
            scb_row[row] = scales[lt0 + lt, 1]
        sca_arr = np.broadcast_to(sca_row[None, :], (128, ROWS)).astype(NPBF16)
        scb_arr = np.broadcast_to(scb_row[None, :], (128, ROWS)).astype(NPBF16)
        sba2_arr = np.zeros((128, 2048), np.float32)
        for row in range(ROWS):
            l = row // RPL
            p = PB_L[l] + (row % RPL)
            blk = BCOL[l]
            sba2_arr[p, blk : blk + 1024] = sca_row[row] * bA2
        sbb2r0_arr = np.zeros((128, 4 * 1024), np.float32)
        sbb2r1_arr = np.zeros((128, 4 * 1024), np.float32)
        for l in range(L):
            for d in range(16):
                for i_ in range(7):
                    sc0 = scales[lt0 + l * 7 + i_ // 2, 1]
                    sc1 = scales[lt0 + l * 7 + (7 + i_) // 2, 1]
                    sbb2r0_arr[d * 7 + i_, l * 1024 : (l + 1) * 1024] = sc0 * bB2
                    sbb2r1_arr[d * 7 + i_, l * 1024 : (l + 1) * 1024] = sc1 * bB2
        in_maps.append(
            {
                "cond": cond_arr,
                "wp": wp_c,
                "bpt2": bpt2_arr,
                "wa1": wa1_arr,
                "wb1": wb1_arr,
                "wa2": wa2_arr,
                "wb2": wb2_arr,
                "ba1": ba1_arr,
                "bb1": bb1_arr,
                "sca": sca_arr,
                "scb": scb_arr,
                "sba2": sba2_arr.astype(NPBF16),
                "sbb2r0": sbb2r0_arr.astype(NPBF16),
                "sbb2r1": sbb2r1_arr.astype(NPBF16),
                "ident": ident_arr,
            }
        )
    return in_maps


def _ensure_ntff_hook():
    """Register the axon NTFF profile hook if the boot didn't (module was
    missing at boot time)."""
    import types

    ah = sys.modules.get("antenv.axon_hooks")
    if ah is None:
        ah = types.ModuleType("antenv.axon_hooks")
        ah._hook = None

        def _set(h, _m=ah):
            _m._hook = h

        def _get(_m=ah):
            return _m._hook

        ah.set_axon_ntff_profile_hook = _set
        ah.get_axon_ntff_profile_hook = _get
        sys.modules["antenv.axon_hooks"] = ah
        import antenv

        antenv.axon_hooks = ah
    if ah.get_axon_ntff_profile_hook() is None:
        if "/root/.axon_site" not in sys.path:
            sys.path.insert(0, "/root/.axon_site")
        from trn_agent_boot.trn_boot import _ntff_profile_via_ctypes

        hook = _ntff_profile_via_ctypes("/opt/axon/libaxon_pjrt.so")
        if hook is not None:
            ah.set_axon_ntff_profile_hook(hook)


def _run(inputs, trace=False):
    if trace:
        _ensure_ntff_hook()
    nc = _get_nc()
    in_maps = _marshal(inputs)
    res = run_bass_kernel_spmd(nc, in_maps, list(range(NCORES)), trace=trace)
    full = np.empty((2, NUM_LAYERS, LAYER_SIZE), np.float32)
    for c in range(NCORES):
        full[:, c * L : (c + 1) * L, :] = res.results[c]["out"].reshape(
            2, L, LAYER_SIZE
        )
    return full.reshape(2, -1), res


def kernel(**inputs) -> np.ndarray:
    out, _ = _run(inputs, trace=False)
    return out
